# revision 1
# baseline (speedup 1.0000x reference)
"""AlphaNet forward pass on 8 Trainium2 NeuronCores (data-parallel over batch).

Pipeline per core (512 samples):
  DVE: rolling-window stats (corr/cov/std/zscore/return/decaylinear) in an
       unscaled "mine" form, written into a padded [272 rows x 16 win] buffer.
  PE : transpose features to [rows, samples]; conv(1x3)+BN folded into small
       sparse matmuls (A pieces) feeding fc1's K-tiles; fc1/fc2/fc3 in bf16.
  ACT: relu/sigmoid + bias epilogues (PSUM f32 -> SBUF bf16).
All per-row constant factors (BatchNorm affine, 1/9, 0.9, 0.3, ret's -1, ...)
are folded into the host-built conv matrix A and per-row bias.
fc1 weight is host-permuted/transposed/bf16-cast and streamed once per core.
"""
import sys
for _p in ("/opt/trn_rl_repo", "/root/.axon_site/_ro/trn_rl_repo"):
    if _p not in sys.path:
        sys.path.append(_p)

from contextlib import ExitStack

import numpy as np
import ml_dtypes

import concourse.bass as bass
import concourse.tile as tile
from concourse import bacc, mybir
from concourse.bass_utils import run_bass_kernel_spmd
from concourse.masks import make_identity

bf16 = ml_dtypes.bfloat16
dt = mybir.dt

# ---- problem constants (hardcoded; must match the AlphaNet reference) ----
NFULL = 4096
NCORES = 8
NSH = NFULL // NCORES        # 512 samples per core
F, W, S = 15, 120, 10
NW = W // S                  # 12
HP = 270                     # stat rows
NROW_PAD, WPAD = 272, 16
GROWS = NROW_PAD * WPAD      # 4352 = 34*128
NGT = GROWS // 128           # 34 transposed-feature tiles
K1 = 43200
K1PAD = 43264                # 338*128
NT = K1PAD // 128            # 338
BN_EPS = 1e-5
NB = NSH // 128              # 4 sample blocks per core


# ------------------------- host-side preparation -------------------------

def _mine_row_tables():
    cb, pairs, base = {}, [], 0
    for d in range(1, 15):
        cb[d] = base
        for i in range(0, 15 - d):
            pairs.append((i, i + d))
        base += 15 - d
    return pairs, cb


def _ref_perm():
    pairs, _ = _mine_row_tables()
    II, JJ = np.triu_indices(F, k=1)
    p2r = {(int(i), int(j)): p for p, (i, j) in enumerate(zip(II, JJ))}
    rom = np.zeros(HP, dtype=np.int64)
    for mh, (i, j) in enumerate(pairs):
        rom[mh] = p2r[(i, j)]
        rom[105 + mh] = 105 + p2r[(i, j)]
    for i in range(15):
        rom[210 + i] = 210 + i
        rom[225 + i] = 225 + i
        rom[240 + i] = 240 + i
        rom[255 + i] = 255 + i
    return rom


def _row_alpha_beta():
    alpha = np.zeros(HP)
    beta = np.zeros(HP)
    alpha[0:105] = 1.0 / 0.9
    alpha[105:210] = 9.0
    alpha[210:225] = 3.0
    alpha[225:240] = 10.0 / 3.0
    alpha[240:255] = 1.0
    beta[240:255] = 1.0
    alpha[255:270] = 1.0
    return alpha, beta


def _conv_pieces():
    plan = []
    for t in range(NT):
        r0, r1 = 128 * t, 128 * t + 127
        h0 = r0 // 160
        hl = min(r1 // 160, HP - 1)
        need = list(range(h0, hl + 1))
        if h0 % 2 == 0:
            pieces = [(h0, need)]
        elif len(need) == 1:
            pieces = [(h0 - 1, [h0])]
        else:
            pieces = [(h0 - 1, [h0]), (h0 + 1, [h0 + 1])]
        plan.append(pieces)
    return plan


def _build_device_inputs(inp):
    gamma = float(inp['bn_gamma'][0]); betab = float(inp['bn_beta'][0])
    mu = float(inp['bn_mean'][0]); var = float(inp['bn_var'][0])
    a = gamma / np.sqrt(var + BN_EPS)
    b = betab - mu * a
    conv_w = np.asarray(inp['conv_w'], np.float64).reshape(16, 3)
    conv_b = np.asarray(inp['conv_b'], np.float64)

    alpha, beta = _row_alpha_beta()
    sA = a / alpha
    sB = b - a * beta / alpha

    ybias = np.zeros(K1PAD, np.float64)
    wsum = conv_w.sum(axis=1)
    for mh in range(HP):
        ybias[mh * 160:(mh + 1) * 160] = np.repeat(conv_b + wsum * sB[mh], 10)
    ybias2d = ybias.reshape(NT, 128).T.astype(np.float32).copy()

    rom = _ref_perm()
    m = np.arange(K1)
    mh = m // 160; o = (m % 160) // 10; wp = m % 10
    kref = o * 2700 + rom[mh] * 10 + wp
    fc1_w = np.asarray(inp['fc1_w'], np.float32)
    fc1t = np.zeros((K1PAD, 512), np.float32)
    fc1t[:K1, :] = fc1_w[:, kref].T
    fc1t = fc1t.astype(bf16)

    # One full-height (K=128) matmul per conv tile against its f-tile u;
    # A streamed from DRAM per tile.
    piece_refs = []
    A_packed = np.zeros((NT, 128, 128), np.float32)
    for t in range(NT):
        r0, r1 = 128 * t, 128 * t + 127
        h0 = r0 // 160
        hl = min(r1 // 160, HP - 1)
        hs = list(range(h0, hl + 1))
        u = h0 // 8                       # f tile holds h in [8u, 8u+8)
        assert all(8 * u <= h < 8 * u + 8 for h in hs), (t, hs, u)
        for col in range(128):
            mm = 128 * t + col
            if mm >= K1:
                continue
            mhh = mm // 160
            if mhh not in hs:
                continue
            oo = (mm % 160) // 10
            wpp = mm % 10
            for k in range(3):
                A_packed[t, (mhh - 8 * u) * 16 + wpp + k, col] = \
                    conv_w[oo, k] * sA[mhh]
        piece_refs.append(u)
    A_packed = A_packed.astype(bf16)
    ncol = 0

    fc1b2d = np.asarray(inp['fc1_b'], np.float32).reshape(4, 128).T.copy()
    fc2t = np.ascontiguousarray(
        np.asarray(inp['fc2_w'], np.float32).T).astype(bf16)
    fc2b = np.asarray(inp['fc2_b'], np.float32).reshape(128, 1).copy()
    fc3t = np.ascontiguousarray(
        np.asarray(inp['fc3_w'], np.float32).reshape(1, 128).T).astype(bf16)
    fc3b = np.asarray(inp['fc3_b'], np.float32).reshape(1, 1).copy()
    wdl = np.tile((np.arange(1, 11, dtype=np.float32) / 55.0)[None, :],
                  (128, 1)).astype(bf16)

    return dict(A_packed=A_packed, fc1t=fc1t, ncol=ncol,
                piece_refs=piece_refs,
                ybias2d=ybias2d, fc1b2d=fc1b2d,
                fc2t=fc2t, fc2b=fc2b, fc3t=fc3t, fc3b=fc3b, wdl=wdl)


# ------------------------- device kernel builder -------------------------

def build_nc(ncol, piece_refs, stage=3):
    nc = bacc.Bacc("TRN2", target_bir_lowering=False, debug=False,
                   num_devices=NCORES)
    f32, b16 = dt.float32, dt.bfloat16
    data_e = nc.declare_dram_parameter("data", [NSH, F * W], b16, isOutput=False)
    A_e = nc.declare_dram_parameter("A_packed", [NT, 128, 128], b16, isOutput=False)
    fc1t_e = nc.declare_dram_parameter("fc1t", [K1PAD, 512], b16, isOutput=False)
    yb_e = nc.declare_dram_parameter("ybias2d", [128, NT], f32, isOutput=False)
    fc1b_e = nc.declare_dram_parameter("fc1b2d", [128, 4], f32, isOutput=False)
    fc2t_e = nc.declare_dram_parameter("fc2t", [512, 128], b16, isOutput=False)
    fc2b_e = nc.declare_dram_parameter("fc2b", [128, 1], f32, isOutput=False)
    fc3t_e = nc.declare_dram_parameter("fc3t", [128, 1], b16, isOutput=False)
    fc3b_e = nc.declare_dram_parameter("fc3b", [1, 1], f32, isOutput=False)
    wdl_e = nc.declare_dram_parameter("wdl", [128, 10], b16, isOutput=False)
    out_e = nc.declare_dram_parameter("out", [1, NSH], f32, isOutput=True)

    _, cb = _mine_row_tables()
    AF = mybir.ActivationFunctionType

    with tile.TileContext(nc) as tc, ExitStack() as ctx:
        consts = ctx.enter_context(tc.tile_pool(name="consts", bufs=1))
        fpool = ctx.enter_context(tc.tile_pool(name="fpool", bufs=1))
        datap = ctx.enter_context(tc.tile_pool(name="datap", bufs=4))
        featp = ctx.enter_context(tc.tile_pool(name="featp", bufs=2))
        spreadp = ctx.enter_context(tc.tile_pool(name="spreadp", bufs=2))
        meansump = ctx.enter_context(tc.tile_pool(name="meansump", bufs=2))
        rstdp = ctx.enter_context(tc.tile_pool(name="rstdp", bufs=2))
        scratch = ctx.enter_context(tc.tile_pool(name="scratch", bufs=4))
        xpool = ctx.enter_context(tc.tile_pool(name="xpool", bufs=4))
        w1pool = ctx.enter_context(tc.tile_pool(name="w1pool", bufs=8))
        apool = ctx.enter_context(tc.tile_pool(name="apool", bufs=8))
        x2pool = ctx.enter_context(tc.tile_pool(name="x2pool", bufs=1))
        outp = ctx.enter_context(tc.tile_pool(name="outp", bufs=1))
        ps_fc1 = ctx.enter_context(tc.tile_pool(name="ps_fc1", bufs=1, space="PSUM"))
        ps_conv = ctx.enter_context(tc.tile_pool(name="ps_conv", bufs=2, space="PSUM"))
        ps_tp = ctx.enter_context(tc.tile_pool(name="ps_tp", bufs=2, space="PSUM"))

        # data DMAs first so stats start immediately
        dtiles = []
        for bkl in range(NB):
            d = datap.tile([128, F, NW, S], b16, tag="d", name=f"d{bkl}")
            nc.sync.dma_start(
                d[:], data_e[128 * bkl:128 * (bkl + 1), :]
                .rearrange("p (f nw s) -> p f nw s", f=F, nw=NW))
            dtiles.append(d)

        # constants
        yb_sb = consts.tile([128, NT], f32)
        nc.sync.dma_start(yb_sb[:], yb_e[:])
        fc1b_sb = consts.tile([128, 4], f32)
        nc.sync.dma_start(fc1b_sb[:], fc1b_e[:])
        fc2t_sb = consts.tile([128, 4, 128], b16)
        nc.sync.dma_start(fc2t_sb[:], fc2t_e.rearrange("(kb k) j -> k kb j", k=128))
        fc2b_sb = consts.tile([128, 1], f32)
        nc.sync.dma_start(fc2b_sb[:], fc2b_e[:])
        fc3t_sb = consts.tile([128, 1], b16)
        nc.sync.dma_start(fc3t_sb[:], fc3t_e[:])
        fc3b_sb = consts.tile([1, 1], f32)
        nc.sync.dma_start(fc3b_sb[:], fc3b_e[:])
        wdl_sb = consts.tile([128, 10], b16)
        nc.sync.dma_start(wdl_sb[:], wdl_e[:])
        ident = consts.tile([128, 128], f32)
        make_identity(nc, ident[:])

        # persistent bf16 transposed-feature buffer [row, sample]
        f_sb = fpool.tile([128, NGT, NSH], b16)

        # ---------------- per-block stats + transpose ----------------
        for bkl in range(NB):
            d = dtiles[bkl]
            feat = featp.tile([128, NROW_PAD, WPAD], f32)
            # zero only the padding (w cols 12:16 and rows 270:272)
            nc.vector.memset(feat[:, :, NW:WPAD], 0.0)
            nc.vector.memset(feat[:, HP:NROW_PAD, 0:NW], 0.0)

            meansum = meansump.tile([128, F, NW], f32)
            nc.vector.tensor_reduce(meansum[:], d[:], axis=mybir.AxisListType.X,
                                    op=mybir.AluOpType.add)
            mean = scratch.tile([128, F, NW], b16, tag="s180")
            nc.vector.tensor_scalar_mul(mean[:], meansum[:], 1.0 / S)
            spread = spreadp.tile([128, F, NW, S], b16)
            nc.vector.tensor_sub(
                spread[:], d[:], mean[:, :, :, None].to_broadcast((128, F, NW, S)))

            varsum = scratch.tile([128, F, NW], f32, tag="s180f")
            prod = scratch.tile([128, F, NW, S], b16, tag="prod")
            nc.vector.tensor_mul(prod[:], spread[:], spread[:])
            nc.vector.tensor_reduce(varsum[:], prod[:], axis=mybir.AxisListType.X,
                                    op=mybir.AluOpType.add)
            nc.scalar.activation(feat[:, 210:225, 0:NW], varsum[:], AF.Sqrt,
                                 bias=0.0, scale=1.0)
            rstd = rstdp.tile([128, F, NW], f32)
            nc.vector.reciprocal(rstd[:], feat[:, 210:225, 0:NW])
            # zscore
            nc.vector.tensor_mul(feat[:, 225:240, 0:NW], meansum[:], rstd[:])
            # return: last/first
            recipf = scratch.tile([128, F, NW], f32, tag="s180f")
            nc.vector.reciprocal(recipf[:], d[:, :, :, 0])
            nc.vector.tensor_mul(feat[:, 240:255, 0:NW], d[:, :, :, S - 1], recipf[:])
            # decay-linear
            dlp = scratch.tile([128, F, NW, S], b16, tag="prod")
            nc.vector.tensor_mul(
                dlp[:], d[:], wdl_sb[:, None, None, :].to_broadcast((128, F, NW, S)))
            nc.vector.tensor_reduce(feat[:, 255:270, 0:NW], dlp[:],
                                    axis=mybir.AxisListType.X, op=mybir.AluOpType.add)
            # cov + corr per offset d
            for dd in range(1, 15):
                nf = 15 - dd
                cp = scratch.tile([128, nf, NW, S], b16, tag="prod")
                nc.vector.tensor_mul(cp[:], spread[:, 0:nf], spread[:, dd:15])
                cov_slice = feat[:, 105 + cb[dd]:105 + cb[dd] + nf, 0:NW]
                nc.vector.tensor_reduce(cov_slice, cp[:],
                                        axis=mybir.AxisListType.X,
                                        op=mybir.AluOpType.add)
                rsp = scratch.tile([128, nf, NW], f32, tag="s180f")
                nc.vector.tensor_mul(rsp[:], rstd[:, 0:nf], rstd[:, dd:15])
                nc.vector.tensor_mul(feat[:, cb[dd]:cb[dd] + nf, 0:NW],
                                     cov_slice, rsp[:])

            # transpose this block's features into f_sb
            featf = feat.rearrange("p r w -> p (r w)")
            for u in range(NGT):
                tp = ps_tp.tile([128, 128], f32, tag="tp", name=f"tp{bkl}_{u}")
                nc.tensor.transpose(tp[:],
                                    featf[:, 128 * u:128 * (u + 1)], ident[:])
                nc.scalar.activation(f_sb[:, u, 128 * bkl:128 * (bkl + 1)],
                                     tp[:], AF.Copy, bias=0.0, scale=1.0)

        if stage <= 1:
            out_sb1 = outp.tile([1, NSH], f32)
            nc.vector.tensor_copy(out_sb1[:], f_sb[0:1, 0, :])
            nc.sync.dma_start(out_e[:], out_sb1[:])

        # ---------------- conv + fc1 stream ----------------
        fc1ps = [ps_fc1.tile([128, NSH], f32, tag=f"jb{jb}", name=f"fc1ps{jb}")
                 for jb in range(4)] if stage >= 3 else None
        last_xt = None
        for t in range(NT if stage >= 2 else 0):
            cps = ps_conv.tile([128, NSH], f32, tag="cps", name=f"cps{t}")
            u = piece_refs[t]
            a1 = apool.tile([128, 128], b16)
            nc.sync.dma_start(a1[:], A_e[t])
            nc.tensor.matmul(cps[:], a1[:], f_sb[:, u, :],
                             start=True, stop=True)
            xt = xpool.tile([128, NSH], b16)
            nc.vector.tensor_scalar(xt[:], cps[:], yb_sb[:, t:t + 1], 0.0,
                                    op0=mybir.AluOpType.add,
                                    op1=mybir.AluOpType.max)
            last_xt = xt
            if stage >= 3:
                w1 = w1pool.tile([128, 512], b16)
                nc.sync.dma_start(w1[:], fc1t_e[128 * t:128 * (t + 1), :])
                for jb in range(4):
                    nc.tensor.matmul(fc1ps[jb][:],
                                     w1[:, 128 * jb:128 * (jb + 1)],
                                     xt[:], start=(t == 0), stop=(t == NT - 1))

        if stage == 2:
            out_sb2 = outp.tile([1, NSH], f32)
            nc.vector.tensor_copy(out_sb2[:], last_xt[0:1, :])
            nc.sync.dma_start(out_e[:], out_sb2[:])

        # ---------------- fc2 / fc3 ----------------
        x2 = (x2pool.tile([128, 4, NSH], b16, name="x2")
              if stage >= 3 else None)
        for jb in range(4 if stage >= 3 else 0):
            nc.scalar.activation(x2[:, jb, :], fc1ps[jb][:], AF.Relu,
                                 bias=fc1b_sb[:, jb:jb + 1], scale=1.0)
        if stage >= 3:
            fc2ps = ps_fc1.tile([128, NSH], f32, tag="jb0")
            for kb in range(4):
                nc.tensor.matmul(fc2ps[:], fc2t_sb[:, kb, :], x2[:, kb, :],
                                 start=(kb == 0), stop=(kb == 3))
            x3 = x2pool.tile([128, NSH], b16)
            nc.scalar.activation(x3[:], fc2ps[:], AF.Sigmoid,
                                 bias=fc2b_sb[:], scale=1.0)
            fc3ps = ps_fc1.tile([128, NSH], f32, tag="jb1")
            nc.tensor.matmul(fc3ps[0:1, :], fc3t_sb[:], x3[:],
                             start=True, stop=True)
            out_sb = outp.tile([1, NSH], f32)
            nc.scalar.activation(out_sb[:], fc3ps[0:1, :], AF.Identity,
                                 bias=fc3b_sb[:], scale=1.0)
            nc.sync.dma_start(out_e[:], out_sb[:])

    nc.compile()
    return nc


# ------------------------------- entry -------------------------------

def _prep_in_maps(inputs):
    dev = _build_device_inputs(inputs)
    data = np.ascontiguousarray(
        np.asarray(inputs['data'], np.float32).reshape(NFULL, F * W)).astype(bf16)
    shared = {k: dev[k] for k in ('A_packed', 'fc1t', 'ybias2d', 'fc1b2d',
                                  'fc2t', 'fc2b', 'fc3t', 'fc3b', 'wdl')}
    in_maps = []
    for c in range(NCORES):
        m = dict(shared)
        m['data'] = data[NSH * c:NSH * (c + 1)]
        in_maps.append(m)
    return dev, in_maps


def run(inputs, trace=False, tmpdir=None):
    dev, in_maps = _prep_in_maps(inputs)
    nc = build_nc(dev['ncol'], dev['piece_refs'])
    res = run_bass_kernel_spmd(nc, in_maps, core_ids=list(range(NCORES)),
                               trace=trace, tmpdir=tmpdir)
    out = np.concatenate([np.asarray(r["out"], np.float32).reshape(NSH)
                          for r in res.results])
    return out, res


def kernel(**inputs) -> np.ndarray:
    out, _ = run(inputs, trace=False)
    return out



# revision 9
# speedup vs baseline: 1.8460x; 1.8460x over previous
"""AlphaNet forward pass on 8 Trainium2 NeuronCores (data-parallel over batch).

Per core (512 samples), software-pipelined across engines:
  DVE : window stats (std/zs/dl + per-offset cov/corr) in bf16, in a row
        order chosen so rows complete in K-consumption order.
  DMA : XBAR transposes of finished feature-row ranges into f_sb
        [row, sample]; chunked weight streams (A, fc1).
  PE  : conv(1x3)+BN folded into per-tile A matmuls; fc1 in fp8-e4m3
        DoubleRow (K=256/instr) except the unbounded "return" rows which
        stay bf16; fc2/fc3 bf16.
  ACT/DVE/Pool: conv epilogues relu(cps+bias) -> fp8/bf16, rotated.
Scales: activations x32 (folded into A+ybias), fc1 weights x128 (divided
out at the fc1 epilogue). All host prep is deterministic index math.
"""
import sys
for _p in ("/opt/trn_rl_repo", "/root/.axon_site/_ro/trn_rl_repo"):
    if _p not in sys.path:
        sys.path.append(_p)

from collections import deque
from contextlib import ExitStack

import numpy as np
import ml_dtypes

import concourse.bass as bass
import concourse.tile as tile
from concourse import bacc, mybir
from concourse.bass_utils import run_bass_kernel_spmd

bf16 = ml_dtypes.bfloat16
f8np = ml_dtypes.float8_e4m3fn
dt = mybir.dt

# ---- problem constants ----
NFULL, NCORES = 4096, 8
NSH = NFULL // NCORES            # 512 samples per core
F, W, S = 15, 120, 10
NW = 12
NROW_PAD, WPAD = 272, 16
GROWS = NROW_PAD * WPAD          # 4352
NGT = GROWS // 128               # 34
NP8 = 160                        # fp8 DoubleRow pairs
NT8 = 2 * NP8                    # 320 fp8 K-tiles
K8 = NT8 * 128                   # 40960
K8_REAL = 255 * 160              # 40800
NTR = 19                         # ret (bf16) K-tiles
KRET_REAL = 15 * 160             # 2400
NT = NT8 + NTR                   # 339
K1PAD = NT * 128                 # 43392
BN_EPS = 1e-5
SX = 32.0                        # fp8 activation scale
SW = 128.0                       # fc1 weight scale
PPU = 5                          # fp8 pairs per u (w1 DMA chunk = one u)
NGT8 = 32                        # fp8-section u count (w1 chunks)
TPU = 10                         # K-tiles per u (A DMA chunk = one u)
RETCH = 5                        # ret tiles per chunk
NRCH = (NTR + RETCH - 1) // RETCH  # 4
NB = NSH // 128                  # 4 sample blocks

R_STD, R_ZS, R_DL, R_COV, R_CORR, R_RET = 0, 15, 30, 45, 150, 256
PAD_TILE = 319                   # all-pad fp8 K-tile (no conv emitted)


# ------------------------- host-side preparation -------------------------

def _mine_pairs():
    cb, pairs, base = {}, [], 0
    for d in range(1, 15):
        cb[d] = base
        for i in range(0, 15 - d):
            pairs.append((i, i + d))
        base += 15 - d
    return pairs, cb


def _ref_row_map():
    pairs, _ = _mine_pairs()
    II, JJ = np.triu_indices(F, k=1)
    p2r = {(int(i), int(j)): p for p, (i, j) in enumerate(zip(II, JJ))}
    rref = np.full(NROW_PAD, -1, np.int64)
    for r in range(15):
        rref[R_STD + r] = 210 + r
        rref[R_ZS + r] = 225 + r
        rref[R_DL + r] = 255 + r
        rref[R_RET + r] = 240 + r
    for q, (i, j) in enumerate(pairs):
        rref[R_COV + q] = 105 + p2r[(i, j)]
        rref[R_CORR + q] = p2r[(i, j)]
    return rref


def _row_scales():
    A = np.ones(NROW_PAD)
    C = np.zeros(NROW_PAD)
    A[R_STD:R_STD + 15] = 3.0
    A[R_ZS:R_ZS + 15] = 10.0 / 3.0
    A[R_COV:R_COV + 105] = 9.0
    A[R_CORR:R_CORR + 105] = 10.0 / 9.0
    C[R_RET:R_RET + 15] = 1.0
    return A, C


def _m_to_rowp(m):
    if m < K8:
        if m >= K8_REAL:
            return None
        r = m // 160
    else:
        mm = m - K8
        if mm >= KRET_REAL:
            return None
        r = R_RET + mm // 160
        m = mm
    return r, (m % 160) // 10, m % 10


def _build_device_inputs(inp):
    gamma = float(inp['bn_gamma'][0]); betab = float(inp['bn_beta'][0])
    mu = float(inp['bn_mean'][0]); var = float(inp['bn_var'][0])
    a = gamma / np.sqrt(var + BN_EPS)
    b = betab - mu * a
    conv_w = np.asarray(inp['conv_w'], np.float64).reshape(16, 3)
    conv_b = np.asarray(inp['conv_b'], np.float64)
    wsum = conv_w.sum(axis=1)

    Ar, Cr = _row_scales()
    rref = _ref_row_map()
    sA = a / Ar
    sB = b - a * Cr / Ar

    A_packed = np.zeros((NT, 128, 128), np.float32)
    ybias = np.zeros(K1PAD, np.float64)
    piece_refs = np.zeros(NT, np.int64)
    for t in range(NT):
        us = set()
        scale = SX if t < NT8 else 1.0
        for c in range(128):
            rp = _m_to_rowp(128 * t + c)
            if rp is None:
                continue
            r, o, wp = rp
            u = r // 8
            us.add(u)
            for k in range(3):
                A_packed[t, (r - 8 * u) * 16 + wp + k, c] = \
                    conv_w[o, k] * sA[r] * scale
            ybias[128 * t + c] = (conv_b[o] + wsum[o] * sB[r]) * scale
        assert len(us) <= 1, (t, us)
        piece_refs[t] = us.pop() if us else 31
    ybias2d = np.ascontiguousarray(
        ybias.reshape(NT, 128).T.astype(np.float32))

    fc1_w = np.asarray(inp['fc1_w'], np.float32)
    Wfull = np.zeros((K1PAD, 512), np.float32)
    valid_m = [m for m in range(K1PAD) if _m_to_rowp(m) is not None]
    kref = np.zeros(len(valid_m), np.int64)
    for ii, m in enumerate(valid_m):
        r, o, wp = _m_to_rowp(m)
        kref[ii] = o * 2700 + rref[r] * 10 + wp
    Wfull[valid_m] = fc1_w[:, kref].T

    Wq8 = np.clip(Wfull[:K8] * (SW / SX), -240, 240).astype(f8np)
    # per-u chunks: [u, k, pu, jb, i, j]
    w1q = np.ascontiguousarray(
        Wq8.reshape(NGT8, PPU, 2, 128, 4, 128)
           .transpose(0, 3, 1, 4, 2, 5)
           .reshape(NGT8, 128, PPU * 4 * 2 * 128))
    wret = np.ascontiguousarray(
        (Wfull[K8:] * SW).astype(bf16).reshape(NTR, 128, 512))

    A_ch = np.zeros((NGT, 128, TPU * 128), np.float32)
    for t in range(NT):
        u, j = t // TPU, t % TPU
        A_ch[u, :, j * 128:(j + 1) * 128] = A_packed[t]
    A_ch = A_ch.astype(bf16)

    fc1b2d = np.ascontiguousarray(
        np.asarray(inp['fc1_b'], np.float32).reshape(4, 128).T)
    fc2t = np.ascontiguousarray(
        np.asarray(inp['fc2_w'], np.float32).T).astype(bf16)
    fc2b = np.asarray(inp['fc2_b'], np.float32).reshape(128, 1).copy()
    fc3t = np.ascontiguousarray(
        np.asarray(inp['fc3_w'], np.float32).reshape(1, 128).T).astype(bf16)
    fc3b = np.asarray(inp['fc3_b'], np.float32).reshape(1, 1).copy()
    wdl = np.tile((np.arange(1, 11, dtype=np.float32) / 55.0)[None, :],
                  (128, 1)).astype(bf16)

    return dict(A_ch=A_ch, w1q=w1q, wret=wret, ybias2d=ybias2d,
                fc1b2d=fc1b2d, fc2t=fc2t, fc2b=fc2b, fc3t=fc3t, fc3b=fc3b,
                wdl=wdl, piece_refs=piece_refs)


# ------------------------- device kernel builder -------------------------

# conv epilogue engine choice: ACT-only while DVE is stats-bound,
# then alternate ACT/DVE
EPI_SWITCH = 90


def build_nc(piece_refs, stage=3):
    nc = bacc.Bacc("TRN2", target_bir_lowering=False, debug=False,
                   num_devices=NCORES)
    f32, b16, f8 = dt.float32, dt.bfloat16, dt.float8e4
    AF = mybir.ActivationFunctionType
    ALU = mybir.AluOpType
    AX = mybir.AxisListType.X

    data_e = nc.declare_dram_parameter("data", [NSH, F * W], b16, isOutput=False)
    A_e = nc.declare_dram_parameter("A_ch", [NGT, 128, TPU * 128], b16, isOutput=False)
    w1q_e = nc.declare_dram_parameter("w1q", [NGT8, 128, PPU * 1024], f8, isOutput=False)
    wret_e = nc.declare_dram_parameter("wret", [NTR, 128, 512], b16, isOutput=False)
    yb_e = nc.declare_dram_parameter("ybias2d", [128, NT], f32, isOutput=False)
    fc1b_e = nc.declare_dram_parameter("fc1b2d", [128, 4], f32, isOutput=False)
    fc2t_e = nc.declare_dram_parameter("fc2t", [512, 128], b16, isOutput=False)
    fc2b_e = nc.declare_dram_parameter("fc2b", [128, 1], f32, isOutput=False)
    fc3t_e = nc.declare_dram_parameter("fc3t", [128, 1], b16, isOutput=False)
    fc3b_e = nc.declare_dram_parameter("fc3b", [1, 1], f32, isOutput=False)
    wdl_e = nc.declare_dram_parameter("wdl", [128, 10], b16, isOutput=False)
    out_e = nc.declare_dram_parameter("out", [1, NSH], f32, isOutput=True)

    _, cb = _mine_pairs()
    tiles_for_u = [[] for _ in range(NGT)]
    for t in range(NT):
        if t != PAD_TILE:
            tiles_for_u[piece_refs[t]].append(t)

    with tile.TileContext(nc) as tc, ExitStack() as ctx:
        consts = ctx.enter_context(tc.tile_pool(name="consts", bufs=1))
        dpool = ctx.enter_context(tc.tile_pool(name="dpool", bufs=1))
        featp = ctx.enter_context(tc.tile_pool(name="featp", bufs=1))
        spreadp = ctx.enter_context(tc.tile_pool(name="spreadp", bufs=1))
        rstdp = ctx.enter_context(tc.tile_pool(name="rstdp", bufs=1))
        msump = ctx.enter_context(tc.tile_pool(name="msump", bufs=2))
        scr = ctx.enter_context(tc.tile_pool(name="scr", bufs=2))
        fpool = ctx.enter_context(tc.tile_pool(name="fpool", bufs=1))
        xqp = ctx.enter_context(tc.tile_pool(name="xqp", bufs=8))
        xrp = ctx.enter_context(tc.tile_pool(name="xrp", bufs=6))
        w1p = ctx.enter_context(tc.tile_pool(name="w1p", bufs=3))
        apl = ctx.enter_context(tc.tile_pool(name="apl", bufs=3))
        wrp = ctx.enter_context(tc.tile_pool(name="wrp", bufs=2))
        x2pool = ctx.enter_context(tc.tile_pool(name="x2pool", bufs=1))
        outp = ctx.enter_context(tc.tile_pool(name="outp", bufs=1))
        ps_fc1 = ctx.enter_context(tc.tile_pool(name="ps_fc1", bufs=1, space="PSUM"))
        ps_conv = ctx.enter_context(tc.tile_pool(name="ps_conv", bufs=4, space="PSUM"))

        # ---------------- input + const DMAs ----------------
        dtile = dpool.tile([128, NB, F, NW, S], b16)
        nc.sync.dma_start(
            dtile[:], data_e.rearrange("(nb p) (f nw s) -> p nb f nw s",
                                       nb=NB, f=F, nw=NW))
        yb_sb = consts.tile([128, NT], f32)
        nc.sync.dma_start(yb_sb[:], yb_e[:])
        fc1b_sb = consts.tile([128, 4], f32)
        nc.sync.dma_start(fc1b_sb[:], fc1b_e[:])
        fc2t_sb = consts.tile([128, 4, 128], b16)
        nc.sync.dma_start(fc2t_sb[:], fc2t_e.rearrange("(kb k) j -> k kb j", k=128))
        fc2b_sb = consts.tile([128, 1], f32)
        nc.sync.dma_start(fc2b_sb[:], fc2b_e[:])
        fc3t_sb = consts.tile([128, 1], b16)
        nc.sync.dma_start(fc3t_sb[:], fc3t_e[:])
        fc3b_sb = consts.tile([1, 1], f32)
        nc.sync.dma_start(fc3b_sb[:], fc3b_e[:])
        wdl_sb = consts.tile([128, 10], b16)
        nc.sync.dma_start(wdl_sb[:], wdl_e[:])

        # persistent transposed-feature buffer [row-part, u, sample]
        f_sb = fpool.tile([128, NGT, NSH], b16)

        # ---------------- weight-chunk streaming ----------------
        a_tiles, w1_tiles, wr_tiles = {}, {}, {}

        def ensure_a(u):
            if u in a_tiles or u >= NGT:
                return
            tl = apl.tile([128, TPU, 128], b16, tag="a", name=f"A{u}")
            nc.sync.dma_start(tl[:], A_e[u].rearrange("k (t c) -> k t c", t=TPU))
            a_tiles[u] = tl

        def ensure_w1(u):
            if u in w1_tiles or u >= NGT8:
                return
            tl = w1p.tile([128, PPU, 4, 2, 128], f8, tag="w", name=f"W{u}")
            nc.gpsimd.dma_start(
                tl[:], w1q_e[u].rearrange("k (pu jb i j) -> k pu jb i j",
                                          pu=PPU, jb=4, i=2))
            w1_tiles[u] = tl

        def ensure_wr(c):
            if c in wr_tiles or c >= NRCH:
                return
            n = min(RETCH, NTR - c * RETCH)
            tl = wrp.tile([128, RETCH, 512], b16, tag="r", name=f"R{c}")
            nc.gpsimd.dma_start(
                tl[:, 0:n, :],
                wret_e[c * RETCH:c * RETCH + n].rearrange("t k j -> k t j"))
            wr_tiles[c] = tl

        # ---------------- conv + fc1 emission machinery ----------------
        xq_tiles = {}           # pair -> fp8 [128, 2, 512]
        xr_tiles = {}           # ret tile t -> bf16 [128, 512]
        pending = deque()       # ('p8', pair) or ('ret', t)
        started = [False] * 4
        epi_i = [0]

        def emit_conv(t):
            u = piece_refs[t]
            cps = ps_conv.tile([128, NSH], f32, tag="cps", name=f"cps{t}")
            nc.tensor.matmul(cps[:], a_tiles[u][:, t % TPU, :], f_sb[:, u, :],
                             start=True, stop=True)
            if t < NT8:
                p = t // 2
                if p not in xq_tiles:
                    xq_tiles[p] = xqp.tile([128, 2, NSH], f8, tag="xq",
                                           name=f"xq{p}")
                    if p == NP8 - 1:
                        nc.gpsimd.memset(xq_tiles[p][:, 1, :], 0.0)
                out_ap = xq_tiles[p][:, t % 2, :]
            else:
                xr_tiles[t] = xrp.tile([128, NSH], b16, tag="xr",
                                       name=f"xr{t}")
                out_ap = xr_tiles[t][:]
            i_e = epi_i[0]
            epi_i[0] += 1
            eng = 'a' if i_e < EPI_SWITCH else 'av'[i_e % 2]
            if eng == 'a':
                nc.scalar.activation(out_ap, cps[:], AF.Relu,
                                     bias=yb_sb[:, t:t + 1], scale=1.0)
            else:
                nc.vector.tensor_scalar(out_ap, cps[:], yb_sb[:, t:t + 1], 0.0,
                                        op0=ALU.add, op1=ALU.max)
            if t < NT8:
                if t % 2 == 1 or t == NT8 - 2:
                    pending.append(('p8', t // 2))
            else:
                pending.append(('ret', t))

        fc1ps = [ps_fc1.tile([128, NSH], f32, tag=f"jb{jb}", name=f"fc1ps{jb}")
                 for jb in range(4)]

        def emit_fc1(item, stop=False):
            kind, idx = item
            if kind == 'p8':
                ch, pu = idx // PPU, idx % PPU
                for jb in range(4):
                    nc.tensor.matmul(
                        fc1ps[jb][:], w1_tiles[ch][:, pu, jb, :, :],
                        xq_tiles[idx][:, 0:2, :],
                        start=not started[jb], stop=stop,
                        perf_mode=mybir.MatmulPerfMode.DoubleRow,
                        skip_group_check=True)
                    started[jb] = True
            else:
                tt = idx - NT8
                ch = tt // RETCH
                ensure_wr(ch); ensure_wr(ch + 1)
                for jb in range(4):
                    nc.tensor.matmul(
                        fc1ps[jb][:],
                        wr_tiles[ch][:, tt % RETCH, 128 * jb:128 * (jb + 1)],
                        xr_tiles[idx][:],
                        start=not started[jb], stop=stop,
                        skip_group_check=True)
                    started[jb] = True

        def drain_pending(keep=2):
            while len(pending) > keep:
                emit_fc1(pending.popleft())

        def consume_u(u):
            ensure_a(u)
            if u < NGT8:
                ensure_w1(u)
            k = u_seq0.index(u)
            for un in u_seq0[k + 1:k + 3]:
                ensure_a(un)
                if un < NGT8:
                    ensure_w1(un)
            for t in tiles_for_u[u]:
                emit_conv(t)
                drain_pending()

        # transpose bookkeeping
        rows_done = np.zeros(NROW_PAD, bool)
        u_emitted = np.zeros(NGT, bool)

        # dry pass: static u emission order
        def _static_u_seq():
            rd = np.zeros(NROW_PAD, bool)
            em = np.zeros(NGT, bool)
            seq = []

            def sweep():
                for u in range(NGT):
                    if not em[u] and rd[8 * u:8 * u + 8].all():
                        em[u] = True
                        seq.append(u)

            rd[R_STD:R_DL + 15] = True
            rd[R_COV + 210] = True
            rd[R_RET:R_RET + 15] = True
            rd[NROW_PAD - 1] = True
            sweep()
            for dd2 in range(1, 15):
                nf2 = 15 - dd2
                rd[R_COV + cb[dd2]:R_COV + cb[dd2] + nf2] = True
                rd[R_CORR + cb[dd2]:R_CORR + cb[dd2] + nf2] = True
                sweep()
            assert em.all()
            return seq

        u_seq0 = _static_u_seq()

        def emit_ready_transposes():
            new_us = []
            for u in range(NGT):
                if not u_emitted[u] and rows_done[8 * u:8 * u + 8].all():
                    u_emitted[u] = True
                    new_us.append(u)
            # group contiguous
            ranges = []
            for u in new_us:
                if ranges and ranges[-1][1] == u:
                    ranges[-1][1] = u + 1
                else:
                    ranges.append([u, u + 1])
            for u0, u1 in ranges:
                for b in range(NB):
                    nc.sync.dma_start_transpose(
                        f_sb[:, u0:u1, 128 * b:128 * (b + 1)],
                        feats[b][:, 8 * u0:8 * u1, :])
            if stage >= 2:
                for u0, u1 in ranges:
                    for u in range(u0, u1):
                        consume_u(u)

        # ---------------- stats: prologue per block ----------------
        feats, spreads, rstds = [], [], []
        for b in range(NB):
            feat = featp.tile([128, NROW_PAD, WPAD], b16, tag=f"feat{b}")
            nc.gpsimd.memset(feat[:, :, NW:WPAD], 0.0)
            nc.gpsimd.memset(feat[:, R_COV + 210:R_COV + 211, 0:NW], 0.0)
            nc.gpsimd.memset(feat[:, NROW_PAD - 1:NROW_PAD, 0:NW], 0.0)
            d = dtile[:, b]
            meansum = msump.tile([128, F, NW], f32, tag="ms")
            nc.vector.tensor_reduce(meansum[:], d, axis=AX, op=ALU.add)
            mean16 = scr.tile([128, F, NW], b16, tag="mean")
            nc.vector.tensor_scalar_mul(mean16[:], meansum[:], 1.0 / S)
            spread = spreadp.tile([128, F, NW, S], b16, tag=f"sp{b}")
            nc.vector.tensor_sub(
                spread[:], d, mean16[:, :, :, None].to_broadcast((128, F, NW, S)))
            sq = scr.tile([128, F, NW, S], b16, tag="sq")
            nc.vector.tensor_mul(sq[:], spread[:], spread[:])
            varsum = scr.tile([128, F, NW], f32, tag="vs")
            nc.vector.tensor_reduce(varsum[:], sq[:], axis=AX, op=ALU.add)
            nc.scalar.activation(feat[:, R_STD:R_STD + 15, 0:NW], varsum[:],
                                 AF.Sqrt, bias=0.0, scale=1.0)
            rstd = rstdp.tile([128, F, NW], f32, tag=f"rs{b}")
            nc.vector.reciprocal(rstd[:], feat[:, R_STD:R_STD + 15, 0:NW])
            nc.vector.tensor_mul(feat[:, R_ZS:R_ZS + 15, 0:NW], meansum[:], rstd[:])
            recipf = scr.tile([128, F, NW], f32, tag="rf")
            nc.vector.reciprocal(recipf[:], d[:, :, :, 0])
            nc.gpsimd.tensor_mul(feat[:, R_RET:R_RET + 15, 0:NW],
                                 d[:, :, :, S - 1], recipf[:])
            dlp = scr.tile([128, F, NW, S], b16, tag="dlp")
            nc.gpsimd.tensor_mul(
                dlp[:], d, wdl_sb[:, None, None, :].to_broadcast((128, F, NW, S)))
            dlh = scr.tile([128, F, NW, 5], b16, tag="dlh")
            nc.gpsimd.tensor_add(dlh[:], dlp[:, :, :, 0:5], dlp[:, :, :, 5:10])
            with nc.allow_low_precision(reason="bf16 feature rows"):
                nc.vector.tensor_reduce(feat[:, R_DL:R_DL + 15, 0:NW], dlh[:],
                                        axis=AX, op=ALU.add)
            feats.append(feat); spreads.append(spread); rstds.append(rstd)

        rows_done[R_STD:R_DL + 15] = True
        rows_done[R_COV + 210] = True          # pad row 255
        rows_done[R_RET:R_RET + 15] = True
        rows_done[NROW_PAD - 1] = True         # pad row 271
        if stage >= 2:
            emit_ready_transposes()

        # ---------------- stats: cov/corr loop ----------------
        for dd in range(1, 15):
            nf = 15 - dd
            for b in range(NB):
                spread, rstd, feat = spreads[b], rstds[b], feats[b]
                cp = scr.tile([128, 14, NW, S], b16, tag="cp")
                nc.vector.tensor_mul(cp[:, 0:nf], spread[:, 0:nf],
                                     spread[:, dd:15])
                hh = scr.tile([128, 14, NW, 5], b16, tag="hh")
                nc.gpsimd.tensor_add(hh[:, 0:nf], cp[:, 0:nf, :, 0:5],
                                     cp[:, 0:nf, :, 5:10])
                cov_sl = feat[:, R_COV + cb[dd]:R_COV + cb[dd] + nf, 0:NW]
                with nc.allow_low_precision(reason="bf16 feature rows"):
                    nc.vector.tensor_reduce(cov_sl, hh[:, 0:nf], axis=AX,
                                            op=ALU.add)
                rsp = scr.tile([128, 14, NW], f32, tag="rsp")
                nc.gpsimd.tensor_mul(rsp[:, 0:nf], rstd[:, 0:nf], rstd[:, dd:15])
                nc.vector.tensor_mul(
                    feat[:, R_CORR + cb[dd]:R_CORR + cb[dd] + nf, 0:NW],
                    cov_sl, rsp[:, 0:nf])
            rows_done[R_COV + cb[dd]:R_COV + cb[dd] + nf] = True
            rows_done[R_CORR + cb[dd]:R_CORR + cb[dd] + nf] = True
            if stage >= 2:
                emit_ready_transposes()

        if stage == 1:
            # debug: dump one f_sb row
            rows_done[:] = True
            emit_ready_transposes()

        if stage >= 2:
            assert u_emitted.all(), np.nonzero(~u_emitted)
            while pending:
                emit_fc1(pending.popleft(), stop=(len(pending) == 0))

        # ---------------- fc2 / fc3 tail ----------------
        if stage >= 3:
            x2 = x2pool.tile([128, 4, NSH], b16, name="x2")
            for jb in range(4):
                nc.scalar.activation(x2[:, jb, :], fc1ps[jb][:], AF.Relu,
                                     bias=fc1b_sb[:, jb:jb + 1], scale=1.0 / SW)
            fc2ps = ps_fc1.tile([128, NSH], f32, tag="jb0")
            for kb in range(4):
                nc.tensor.matmul(fc2ps[:], fc2t_sb[:, kb, :], x2[:, kb, :],
                                 start=(kb == 0), stop=(kb == 3))
            x3 = x2pool.tile([128, NSH], b16)
            nc.scalar.activation(x3[:], fc2ps[:], AF.Sigmoid,
                                 bias=fc2b_sb[:], scale=1.0)
            fc3ps = ps_fc1.tile([128, NSH], f32, tag="jb1")
            nc.tensor.matmul(fc3ps[0:1, :], fc3t_sb[:], x3[:],
                             start=True, stop=True)
            out_sb = outp.tile([1, NSH], f32)
            nc.scalar.activation(out_sb[:], fc3ps[0:1, :], AF.Identity,
                                 bias=fc3b_sb[:], scale=1.0)
            nc.sync.dma_start(out_e[:], out_sb[:])
        elif stage == 1:
            out_sb1 = outp.tile([1, NSH], f32)
            nc.vector.tensor_copy(out_sb1[:], f_sb[0:1, 0, :])
            nc.sync.dma_start(out_e[:], out_sb1[:])
        elif stage == 2:
            out_sb2 = outp.tile([1, NSH], f32)
            nc.scalar.activation(out_sb2[:], fc1ps[0][0:1, :], AF.Identity,
                                 bias=0.0, scale=1.0)
            nc.sync.dma_start(out_e[:], out_sb2[:])

    nc.compile()
    return nc


# ------------------------------- entry -------------------------------

def _prep_in_maps(inputs):
    dev = _build_device_inputs(inputs)
    data = np.ascontiguousarray(
        np.asarray(inputs['data'], np.float32).reshape(NFULL, F * W)).astype(bf16)
    shared = {k: dev[k] for k in ('A_ch', 'w1q', 'wret', 'ybias2d', 'fc1b2d',
                                  'fc2t', 'fc2b', 'fc3t', 'fc3b', 'wdl')}
    in_maps = []
    for c in range(NCORES):
        m = dict(shared)
        m['data'] = data[NSH * c:NSH * (c + 1)]
        in_maps.append(m)
    return dev, in_maps


def run(inputs, trace=False, tmpdir=None, stage=3):
    dev, in_maps = _prep_in_maps(inputs)
    nc = build_nc(dev['piece_refs'], stage=stage)
    res = run_bass_kernel_spmd(nc, in_maps, core_ids=list(range(NCORES)),
                               trace=trace, tmpdir=tmpdir)
    out = np.concatenate([np.asarray(r["out"], np.float32).reshape(NSH)
                          for r in res.results])
    return out, res


def kernel(**inputs) -> np.ndarray:
    out, _ = run(inputs, trace=False)
    return out
